# revision 1
# baseline (speedup 1.0000x reference)
"""Trainium2 Bass kernel for nn_BlockSA (Swin-style window attention).

Sharding: data-parallel over batch — 16 images / 8 cores = 2 images per core.
Weights, bias table, and identity constants are replicated (packed into one
const blob -> single DMA).

Per-core program (SPMD, no collectives): 32 chunks, each one row-band
(image i, window-row hw) = 784 tokens = 16 windows:
  1. DMA x band [784, 128] -> xnat [98, 8x128] (pair-blocked).
  2. PE transpose (x8) -> xT [128, 784] (raster order).
  3. Q^T / K^T projections with window-gather APs -> window-major [128, 784];
     K^T twice with zero-padded weights (wkA/wkB) -> block-diag kblk
     [32-row-group, 113-blocks] (gaps pre-zeroed per chunk).
  4. V projection per window, column-split (tile_position (0,0)/(0,64)) ->
     vplus [128, 16x68] with interleaved ones columns (denominator trick).
  5. S^T = blockdiag(K^T).T @ Q^T on 32-row PE tiles (one PSUM bank per
     row-group), + relative-position bias via K-split (32+17) accumulation
     matmuls against replicated identities.
  6. exp on ACT (PSUM->SBUF), no max-subtraction (logits are tiny).
  7. O = P^T.T @ [V | ones] on 64x64 PE tiles -> per-window [49, 4x17]
     blocks; col 16 of each block = softmax denominator.
  8. reciprocal + broadcast-AP multiply -> normalized o [i, c] halves.
  9. PE transpose halves -> o_cat^T [128, 784] window-major.
 10. w_o projection with raster un-gather AP (per pixel-row, M=112) ->
     natural [token, c] output + b_o, DMA store.
"""
import sys
sys.path.insert(0, "/opt/trn_rl_repo")
import numpy as np

WS, NH, C, HS = 7, 8, 128, 16
N = WS * WS            # 49
M113 = 113
B, H, W = 16, 112, 112
NCORES = 8
B_PER_CORE = B // NCORES           # 2
NBANDS = H // WS                   # 16 bands per image
TOK_BAND = WS * W                  # 784 tokens per band
NWIN = W // WS                     # 16 windows per band
TOK_CORE = B_PER_CORE * H * W      # 25088

# const blob column layout
_COLS = dict(wq=C, wkA=C, wkB=C, wv=C, wo=C, biasA=M113, biasB=M113,
             itop8=8 * N, ibot8=8 * N, i49=N, i112=112, bo=C)
CB_TOTAL = sum(_COLS.values())

_CACHE = {}


def _build_module(reps=1):
    import concourse.bass as bass
    import concourse.mybir as mybir
    import concourse.tile as tile
    from concourse import bacc
    from contextlib import ExitStack

    F32 = mybir.dt.float32
    nc = bacc.Bacc(None)
    xin = nc.declare_dram_parameter("xin", [TOK_CORE, C], F32, isOutput=False)
    cblob = nc.declare_dram_parameter("cblob", [C, CB_TOTAL], F32, isOutput=False)
    out = nc.declare_dram_parameter("out", [TOK_CORE, C], F32, isOutput=True)

    with tile.TileContext(nc) as tc, ExitStack() as ctx:
        singles = ctx.enter_context(tc.tile_pool(name="singles", bufs=1))
        sb = ctx.enter_context(tc.tile_pool(name="sb", bufs=2))
        ps = ctx.enter_context(tc.tile_pool(name="ps", bufs=4, space="PSUM"))

        cb = singles.tile([C, CB_TOTAL], F32, tag="cblob", name="cblob_t")
        nc.sync.dma_start(cb[:], cblob[:])
        ofs = {}
        o = 0
        for k, w_ in _COLS.items():
            ofs[k] = o
            o += w_

        def cs(key, p0=0, p1=C, c0=0, c1=None):
            c1 = _COLS[key] if c1 is None else c1
            return cb[p0:p1, ofs[key] + c0:ofs[key] + c1]

        # dummy first-touch matmul to absorb the cblob DMA wait on PE
        dummy_ps = ps.tile([C, 1], F32, tag="pp", name="dummy_ps")
        nc.tensor.matmul(dummy_ps[:, 0:1], lhsT=cs("wq"), rhs=cs("wq", c1=1),
                         start=True, stop=True)

        rep_cm = tc.For_i(0, reps, 1) if reps > 1 else None
        if rep_cm is not None:
            rep_cm.__enter__()

        for chunk in range(B_PER_CORE * NBANDS):
            img, band = divmod(chunk, NBANDS)
            base = img * H * W + band * TOK_BAND
            cn = f"c{chunk}"

            # ---- 1. load x band raster-row-major: xnat [112, 7*128] ----
            xnat = sb.tile([112, WS * C], F32, tag="xnat", name=f"xnat_{cn}")
            xi = xin[:]
            src = bass.AP(tensor=xi.tensor, offset=xi.offset + base * C,
                          ap=[[C, 112], [112 * C, WS], [1, C]])
            nc.sync.dma_start(xnat[:], src)

            # ---- 2. transpose rows -> xt_sb [128, 784] WINDOW-major ----
            xt_p = [ps.tile([C, 448], F32, tag="pp", name=f"xtp{t}_{cn}")
                    for t in range(2)]
            for r in range(WS):
                t_, rl = (0, r) if r < 4 else (1, r - 4)
                nc.tensor.matmul(xt_p[t_][:, 112 * rl:112 * (rl + 1)],
                                 lhsT=xnat[0:112, C * r:C * (r + 1)],
                                 rhs=cs("i112", 0, 112),
                                 start=True, stop=True)
            xt_sb = sb.tile([C, TOK_BAND], F32, tag="xt", name=f"xt_{cn}")
            xta = xt_sb[:]
            for t_, nr, r0 in ((0, 4, 0), (1, 3, 4)):
                # in (r, w, s) raster -> out col 49w + 7r + s
                dst = bass.AP(tensor=xta.tensor, offset=xta.offset + 7 * r0,
                              ap=[list(xta.ap[0]), [7, nr], [49, 16], [1, 7]])
                sv = xt_p[t_].rearrange("p (r ws) -> p r ws", r=4)[:, 0:nr, :]
                nc.vector.tensor_copy(dst, sv.rearrange(
                    "p r (w s) -> p r w s", w=16))

            # ---- 3. Q^T / K^T projections (window-major out) ----
            qt_sb = sb.tile([C, TOK_BAND], F32, tag="qt", name=f"qt_{cn}")
            kblk = sb.tile([C, NWIN * M113], F32, tag="kblk", name=f"kblk_{cn}")
            # zero the gap columns (49:64) of each 113-block
            ka = kblk[:]
            gap = bass.AP(tensor=ka.tensor, offset=ka.offset + 49,
                          ap=[list(ka.ap[0]), [M113, NWIN], [1, 15]])
            nc.vector.memset(gap, 0.0)
            for half in range(2):
                qp = ps.tile([C, 392], F32, tag="pp", name=f"qp{half}_{cn}")
                nc.tensor.matmul(qp[:], lhsT=cs("wq"),
                                 rhs=xt_sb[:, 392 * half:392 * (half + 1)],
                                 start=True, stop=True)
                nc.vector.tensor_copy(qt_sb[:, 392 * half:392 * (half + 1)], qp[:])
                for key, c0 in (("wkA", 0), ("wkB", 64)):
                    kp = ps.tile([C, 392], F32, tag="pp", name=f"kp{key}{half}_{cn}")
                    nc.tensor.matmul(kp[:], lhsT=cs(key),
                                     rhs=xt_sb[:, 392 * half:392 * (half + 1)],
                                     start=True, stop=True)
                    dst = bass.AP(tensor=ka.tensor,
                                  offset=ka.offset + (8 * half) * M113 + c0,
                                  ap=[list(ka.ap[0]), [M113, 8], [1, N]])
                    srcv = kp.rearrange("p (w n) -> p w n", w=8)
                    nc.vector.tensor_copy(dst, srcv)

            # ---- 4. V projection + vplus [C, NWIN*136] ----
            # window block = 4 g-blocks of 34: cols 0:16 = V_g (rows 0:49),
            # col 16 = ones (rows 0:49), cols 17:33 = V_{g+4} (rows 64:113),
            # col 33 = ones (rows 64:113); everything else zero.
            vplus = sb.tile([C, NWIN * 136], F32, tag="vplus", name=f"vplus_{cn}")
            nc.vector.memset(vplus[:], 0.0)
            va = vplus[:]
            ones_top = bass.AP(tensor=va.tensor, offset=va.offset + 32,
                               ap=[[va.ap[0][0], N], [136, NWIN], [34, 4]])
            nc.vector.memset(ones_top, 1.0)
            ones_bot = bass.AP(tensor=va.tensor,
                               offset=va.offset + 64 * va.ap[0][0] + 33,
                               ap=[[va.ap[0][0], N], [136, NWIN], [34, 4]])
            nc.vector.memset(ones_bot, 1.0)
            vv = vplus.rearrange("p (w g s) -> p w g s", w=NWIN, g=4)
            for vq in range(2):  # 8 windows per psum tile, 64 cols each
                vp = ps.tile([C, 512], F32, tag="pp", name=f"vp{vq}_{cn}")
                for wl in range(8):
                    w_ = 8 * vq + wl
                    xg = xt_sb[:, N * w_:N * (w_ + 1)]
                    nc.tensor.matmul(vp[0:N, 64 * wl:64 * wl + 64],
                                     lhsT=xg, rhs=cs("wv", c0=0, c1=64),
                                     start=True, stop=True, tile_position=(0, 0))
                    nc.tensor.matmul(vp[64:64 + N, 64 * wl:64 * wl + 64],
                                     lhsT=xg, rhs=cs("wv", c0=64, c1=128),
                                     start=True, stop=True, tile_position=(0, 64))
                vpv = vp.rearrange("p (w g s) -> p w g s", w=8, g=4)
                nc.vector.tensor_copy(vv[0:N, 8 * vq:8 * vq + 8, :, 0:HS],
                                      vpv[0:N, :, :, :])
                nc.vector.tensor_copy(vv[64:64 + N, 8 * vq:8 * vq + 8, :, HS:2 * HS],
                                      vpv[64:64 + N, :, :, :])

            # ---- 5+6. S^T + bias, exp ----
            p_sb = sb.tile([C, NWIN * 4 * N], F32, tag="psb", name=f"psb_{cn}")
            for half in range(2):
                s_ps = ps.tile([C, 2048], F32, tag="sps", name=f"sps{half}_{cn}",
                               bufs=1)
                for g in range(4):
                    tp = (32 * g, 0)
                    for wl in range(8):
                        w_ = 8 * half + wl
                        nc.tensor.matmul(
                            s_ps[0:M113, 512 * g + N * wl:512 * g + N * (wl + 1)],
                            lhsT=kblk[32 * g:32 * (g + 1),
                                      M113 * w_:M113 * (w_ + 1)],
                            rhs=qt_sb[32 * g:32 * (g + 1), N * w_:N * (w_ + 1)],
                            start=(wl == 0), stop=False, tile_position=tp)
                    nc.tensor.matmul(s_ps[0:M113, 512 * g:512 * g + 392],
                                     lhsT=cs("biasA", 32 * g, 32 * (g + 1)),
                                     rhs=cs("itop8", 32 * g, 32 * (g + 1)),
                                     start=False, stop=False, tile_position=tp)
                    nc.tensor.matmul(s_ps[0:M113, 512 * g:512 * g + 392],
                                     lhsT=cs("biasB", 32 * g, 32 * g + 17),
                                     rhs=cs("ibot8", 32 * g, 32 * g + 17),
                                     start=False, stop=True, tile_position=tp)
                # one exp instruction per half: in (g, w, n), out (w, g, n)
                pa = p_sb[:]
                sa = s_ps[:]
                src_ap = bass.AP(tensor=sa.tensor, offset=sa.offset,
                                 ap=[[sa.ap[0][0], M113], [512, 4], [N, 8], [1, N]])
                dst_ap = bass.AP(tensor=pa.tensor,
                                 offset=pa.offset + (4 * (8 * half)) * N,
                                 ap=[[pa.ap[0][0], M113], [N, 4], [4 * N, 8], [1, N]])
                nc.scalar.activation(dst_ap, src_ap,
                                     mybir.ActivationFunctionType.Exp)

            # ---- 7+8. O matmuls (K=113, both heads per MM) + normalize.
            # o_ps partitions: window parity (even win -> rows 0:49, odd ->
            # 64:113); 34-col block per (pair, g) = [O_g(16)|s_g|O_g4(16)|s_g4].
            # onorm2 [C, 8*128]: pair-major 128-chan blocks, same parity rows.
            onorm = sb.tile([C, 8 * C], F32, tag="onorm", name=f"onorm_{cn}")
            ona = onorm[:]
            pg_sizes = ((0, 3), (3, 6), (6, 8))  # pair-groups
            for pg0, pg1 in pg_sizes:
                npair = pg1 - pg0
                o_ps = ps.tile([C, 136 * npair], F32, tag="pp",
                               name=f"ops{pg0}_{cn}")
                for pl in range(npair):
                    for wl in range(2):
                        w_ = 2 * (pg0 + pl) + wl
                        b0 = 64 * wl
                        for g in range(4):
                            scol = (4 * w_ + g) * N
                            nc.tensor.matmul(
                                o_ps[b0:b0 + N, 136 * pl + 34 * g:
                                     136 * pl + 34 * (g + 1)],
                                lhsT=p_sb[0:M113, scol:scol + N],
                                rhs=vplus[0:M113, 136 * w_ + 34 * g:
                                          136 * w_ + 34 * (g + 1)],
                                start=True, stop=True, tile_position=(0, b0))
                recip = sb.tile([C, 128 * 3], F32, tag="recip",
                                name=f"rc{pg0}_{cn}")
                oa = o_ps[:]
                ra = recip[:]
                for wl in range(2):
                    b0 = 64 * wl
                    pp_o = oa.ap[0][0]
                    pp_r = ra.ap[0][0]
                    # reciprocal of denominators, replicated 16-wide:
                    # out col 32*(4*pl+g) + 16*hh + d  <-  block col 32+hh
                    den = bass.AP(tensor=oa.tensor,
                                  offset=oa.offset + b0 * pp_o + 32,
                                  ap=[[pp_o, N], [34, 4 * npair], [1, 2],
                                      [0, HS]])
                    rc = bass.AP(tensor=ra.tensor, offset=ra.offset + b0 * pp_r,
                                 ap=[[pp_r, N], [1, 128 * npair]])
                    nc.vector.reciprocal(rc, den)
                    src_o = bass.AP(tensor=oa.tensor, offset=oa.offset + b0 * pp_o,
                                    ap=[[pp_o, N], [136, npair], [34, 4],
                                        [1, 2 * HS]])
                    rb = bass.AP(tensor=ra.tensor, offset=ra.offset + b0 * pp_r,
                                 ap=[[pp_r, N], [128, npair], [32, 4],
                                     [1, 2 * HS]])
                    dst_o = bass.AP(tensor=ona.tensor,
                                    offset=ona.offset + b0 * ona.ap[0][0]
                                    + C * pg0,
                                    ap=[[ona.ap[0][0], N], [C, npair], [32, 4],
                                        [1, 2 * HS]])
                    nc.vector.tensor_tensor(dst_o, src_o, rb,
                                            op=mybir.AluOpType.mult)

            # ---- 9. transpose o -> ot_sb [128, 784] raster (1 MM per window,
            # 64-row tiles by window parity; separate banks per row-tile) ----
            ot_sb = sb.tile([C, TOK_BAND], F32, tag="ot", name=f"ot_{cn}")
            ota = ot_sb[:]
            otE = ps.tile([C, 392], F32, tag="pp", name=f"otE_{cn}")
            otO = ps.tile([C, 392], F32, tag="pp", name=f"otO_{cn}")
            for w_ in range(NWIN):
                wl, pair = w_ % 2, w_ // 2
                b0 = 64 * wl
                opst = otE if wl == 0 else otO
                nc.tensor.matmul(opst[:, N * pair:N * (pair + 1)],
                                 lhsT=onorm[b0:b0 + N, C * pair:C * (pair + 1)],
                                 rhs=cs("i49", b0, b0 + N),
                                 start=True, stop=True, tile_position=(b0, 0))
            for wl, src_t in ((0, otE), (1, otO)):
                # psum col (pair, i=(r,s)) -> ot_sb col 112r + 7(2*pair+wl) + s
                dst = bass.AP(tensor=ota.tensor, offset=ota.offset + 7 * wl,
                              ap=[list(ota.ap[0]), [14, 8], [112, 7], [1, 7]])
                sv = src_t.rearrange("p (w r s) -> p w r s", w=8, r=7)
                nc.vector.tensor_copy(dst, sv)

            # ---- 10. w_o projection per raster row (M=112) + b_o; plain store
            f_ps = [ps.tile([C, 512], F32, tag="pp", name=f"fp{t}_{cn}")
                    for t in range(2)]
            for r in range(WS):
                t_, rl = (0, r) if r < 4 else (1, r - 4)
                nc.tensor.matmul(f_ps[t_][0:112, 128 * rl:128 * (rl + 1)],
                                 lhsT=ot_sb[:, 112 * r:112 * (r + 1)],
                                 rhs=cs("wo"),
                                 start=True, stop=True, tile_position=(0, 0))
            fin = sb.tile([C, 896], F32, tag="fin", name=f"fin_{cn}")
            ba = cb[:]
            for t_, nr in ((0, 4), (1, 3)):
                bo_b = bass.AP(tensor=ba.tensor, offset=ba.offset + ofs["bo"],
                               ap=[[ba.ap[0][0], 112], [0, nr], [1, C]])
                nc.vector.tensor_tensor(fin[0:112, 512 * t_:512 * t_ + 128 * nr],
                                        f_ps[t_][0:112, 0:128 * nr], bo_b,
                                        op=mybir.AluOpType.add)
            # store: fin [112-part (raster col), r-blocks] -> DRAM rows
            oap = out[:]
            fa = fin[:]
            dst = bass.AP(tensor=oap.tensor, offset=oap.offset + base * C,
                          ap=[[C, 112], [112 * C, WS], [1, C]])
            s2 = bass.AP(tensor=fa.tensor, offset=fa.offset,
                         ap=[[fa.ap[0][0], 112], [C, WS], [1, C]])
            nc.sync.dma_start(dst, s2)

        if rep_cm is not None:
            rep_cm.__exit__(None, None, None)

    nc.finalize()
    return nc


def _head_perm():
    perm = np.zeros(C, np.int64)
    for g in range(4):
        perm[32 * g:32 * g + 16] = np.arange(16) + 16 * g
        perm[32 * g + 16:32 * g + 32] = np.arange(16) + 16 * (g + 4)
    return perm


def _rel_index():
    coords = np.stack(np.meshgrid(np.arange(WS), np.arange(WS),
                                  indexing="ij"), 0).reshape(2, -1)
    rel = coords[:, :, None] - coords[:, None, :] + (WS - 1)
    return rel[0] * (2 * WS - 1) + rel[1]   # [N, N]


def _build_cblob(w_q, w_k, w_v, w_o, b_o, rel_bias):
    scale = HS ** -0.5
    perm = _head_perm()
    wq_dev = (w_q * scale)[:, perm].astype(np.float32)
    wk_dev = w_k[:, perm].astype(np.float32)
    wkA = wk_dev.copy()
    wkB = wk_dev.copy()
    for g in range(4):
        wkA[:, 32 * g + 16:32 * g + 32] = 0.0
        wkB[:, 32 * g:32 * g + 16] = 0.0

    bias = rel_bias[_rel_index()].transpose(2, 0, 1).astype(np.float32)  # [NH,N,N]
    biasA = np.zeros((C, M113), np.float32)
    biasB = np.zeros((C, M113), np.float32)
    for g in range(4):
        biasA[32 * g:32 * g + 32, 0:N] = bias[g, 0:32, :]
        biasA[32 * g:32 * g + 32, 64:64 + N] = bias[g + 4, 0:32, :]
        biasB[32 * g:32 * g + 17, 0:N] = bias[g, 32:49, :]
        biasB[32 * g:32 * g + 17, 64:64 + N] = bias[g + 4, 32:49, :]

    itop8 = np.zeros((C, 8 * N), np.float32)
    ibot8 = np.zeros((C, 8 * N), np.float32)
    for g in range(4):
        for w_ in range(8):
            itop8[32 * g:32 * g + 32, N * w_:N * w_ + 32] = np.eye(32)
            ibot8[32 * g:32 * g + 17, N * w_ + 32:N * w_ + 49] = np.eye(17)
    i49 = np.zeros((C, N), np.float32)
    for b0 in (0, 64):
        i49[b0:b0 + N, :] = np.eye(N)
    i112 = np.zeros((C, 112), np.float32)
    i112[0:112, :] = np.eye(112)
    bo_rep = np.broadcast_to(np.asarray(b_o, np.float32), (C, C))

    # onorm channel slot 32g+16hh+d holds logical channel 16(g+4hh)+d
    perm2 = np.zeros(C, np.int64)
    for g in range(4):
        for hh in range(2):
            perm2[32 * g + 16 * hh:32 * g + 16 * hh + 16] = \
                np.arange(16) + 16 * (g + 4 * hh)
    wo_dev = np.asarray(w_o, np.float32)[perm2, :]
    parts = dict(wq=wq_dev, wkA=wkA, wkB=wkB, wv=np.asarray(w_v, np.float32),
                 wo=wo_dev, biasA=biasA, biasB=biasB,
                 itop8=itop8, ibot8=ibot8, i49=i49, i112=i112, bo=bo_rep)
    blob = np.concatenate([np.ascontiguousarray(parts[k], dtype=np.float32)
                           for k in _COLS], axis=1)
    assert blob.shape == (C, CB_TOTAL)
    return np.ascontiguousarray(blob)


def kernel(x, w_q, w_k, w_v, w_o, b_o, rel_bias):
    from concourse.bass_utils import run_bass_kernel_spmd

    import os
    x = np.asarray(x, np.float32)
    reps = int(os.environ.get("BLOCKSA_REPS", "1"))
    key = f"nc{reps}"
    if key not in _CACHE:
        _CACHE[key] = _build_module(reps)
    nc = _CACHE[key]

    cblob = _build_cblob(np.asarray(w_q, np.float32), np.asarray(w_k, np.float32),
                         np.asarray(w_v, np.float32), np.asarray(w_o, np.float32),
                         np.asarray(b_o, np.float32),
                         np.asarray(rel_bias, np.float32))

    in_maps = []
    for c in range(NCORES):
        shard = x[B_PER_CORE * c:B_PER_CORE * (c + 1)].reshape(TOK_CORE, C)
        in_maps.append(dict(xin=np.ascontiguousarray(shard), cblob=cblob))

    import os
    trace = os.environ.get("BLOCKSA_TRACE", "0") == "1"
    res = run_bass_kernel_spmd(nc, in_maps, list(range(NCORES)), trace=trace)
    if trace:
        _CACHE["last_result"] = res
        print(f"HW exec time: {res.exec_time_ns} ns", flush=True)
    outs = [res.results[c]["out"].reshape(B_PER_CORE, H, W, C)
            for c in range(NCORES)]
    return np.concatenate(outs, axis=0)

